# revision 30
# baseline (speedup 1.0000x reference)
"""DiffAugment (color jitter + translation + cutout) Trainium2 Bass kernel.

Strategy (data parallel over batch, 16 samples per core on 8 cores), fp16
end-to-end to halve DMA traffic (inputs cast fp32->fp16 during the SWDGE
load; the DRAM output tensor is fp16 and the host widens back to fp32):

  - Color math refactored so the per-pixel work is a plain tensor add:
        y_c = x_c + t',   t' = (Bp/As)*mc3 + D',  D' = (Cp/As)*S + b/As
    with As = max(A, eps). The missing factor A is folded into the H-shift
    matrix (its 0/1 entries become As via a fused is_equal*mult), so
        z_c = (As*SH) @ y_c = A*SH@x_c + SH@(Bp*mc3 + Cp*S + b)
    which matches the reference color+translate result exactly (up to the
    A->As clamp, which changes the x-term by at most eps*|x|).
  - mc3 = x0+x1 on GpSimd (stt), +x2 on DVE (tensor_add).
  - S (total sample sum) via free PE matmuls: 4 column-sum matmuls of mc3
    chunks into PSUM [128,1], evicted to SBUF, then a ones-matmul broadcast.
  - Translation: H (partition-dim) shift via the As-scaled 0/1 shift-matrix
    matmul on TensorE (built on-device from a pinned 3D iota + fused
    is_equal*mult); W (free-dim) shift via a dynamic-start slice when
    evicting PSUM (zero-bordered WPAD layout).
  - Cutout: per-sample row indicators are precomputed on host ([P, 16, 2]
    table); <=51-column band multiplies on DVE (two static-width bands whose
    union is exactly the cut range) applied one sample behind to keep DVE
    from head-of-line blocking on the ACT evictions.
  - Loads are paired (2 samples per SWDGE DMA) into one static x buffer so
    the load stream never waits on compute; stores go on the SP HWDGE ring.
"""

import sys

if "/opt/trn_rl_repo" not in sys.path:
    sys.path.insert(0, "/opt/trn_rl_repo")

import numpy as np

import concourse.bass as bass
import concourse.bacc as bacc
import concourse.tile as tile
import concourse.mybir as mybir
from concourse import bass_utils

F32 = mybir.dt.float32
F16 = mybir.dt.float16
I32 = mybir.dt.int32
AF = mybir.ActivationFunctionType
OP = mybir.AluOpType
ET = mybir.EngineType

N_CORES = 8
B = 128
B_LOC = B // N_CORES  # 16
C, H, W = 3, 256, 256
KT = 2          # number of 128-row partition tiles per image
P = 128
WPAD = W + 64   # W-padded free dim (32 zero cols each side)
PADL = 32
SHIFT = 32      # int(H * 0.125 + 0.5)
CUT = 51        # int(H * 0.2 + 0.5)
BW1, BW2 = 25, 26   # static fixup band widths (union covers any 26..51 range)
A_EPS = 1e-3

# scalar table columns (f32 block, then int32-bit-cast block)
(SC_TX, SC_AS, SC_AP, SC_CPA, SC_BA,
 SC_TYOFF, SC_CY0, SC_CY15) = range(8)
NSCAL = 8

_CACHE = {}


def build_nc():
    """Build + compile the per-core Bass program (cached)."""
    if "nc" in _CACHE:
        return _CACHE["nc"]

    nc = bacc.Bacc(
        "TRN2",
        target_bir_lowering=False,
        debug=False,
        enable_asserts=True,
        num_devices=N_CORES,
    )
    x_d = nc.dram_tensor("x", [B_LOC, C, H, W], F16, kind="ExternalInput").ap()
    scal_d = nc.dram_tensor("scal", [B_LOC, NSCAL], F32, kind="ExternalInput").ap()
    rinv_d = nc.dram_tensor("rinv", [P, B_LOC, KT], F32, kind="ExternalInput").ap()
    out_d = nc.dram_tensor("out", [B_LOC, C, H, W], F16, kind="ExternalOutput").ap()

    with tile.TileContext(nc) as tc:
        _kernel_body(tc, nc, x_d, scal_d, rinv_d, out_d)

    nc.compile()
    _CACHE["nc"] = nc
    return nc


def _kernel_body(tc, nc, x_d, scal_d, rinv_d, out_d):
    NY = 6  # y_pad rotation depth (borders memset once)

    with (
        tc.tile_pool(name="consts", bufs=1) as consts,
        tc.tile_pool(name="ypads", bufs=NY) as ypads,
        tc.tile_pool(name="xt", bufs=10) as xp,
        tc.tile_pool(name="mc3a", bufs=6) as map_,
        tc.tile_pool(name="mc3", bufs=6) as mcp,
        tc.tile_pool(name="cmp", bufs=6) as cmpp,
        tc.tile_pool(name="zt", bufs=6) as zp,
        tc.tile_pool(name="sA", bufs=6) as sap,
        tc.tile_pool(name="Dt", bufs=6) as dp,
        tc.tile_pool(name="pz", bufs=2, space="PSUM") as pzp,
        tc.tile_pool(name="ptiny", bufs=2, space="PSUM") as ptp,
    ):
        # ---- constants ----
        scal_sb = consts.tile([P, B_LOC, NSCAL], F32)
        scal_bcast = bass.AP(
            tensor=scal_d.tensor,
            offset=scal_d.offset,
            ap=[[0, P]] + list(scal_d.ap),
        )
        rinv_sb = consts.tile([P, B_LOC, KT], F32)

        def sc(s, col):  # [128,1] per-sample scalar broadcast column
            return scal_sb[:, s, col:col + 1]

        # The host passes x already in fp16, so loads ride the HWDGE sync
        # ring (SP) and are all issued up front; the pool sem keeps the
        # stream well ahead of compute.
        x_tiles = []
        for s in range(B_LOC):
            x_t = xp.tile([P, C, KT, W], F16)
            src = x_d[s].rearrange("c (kt p) w -> p c kt w", p=P)
            nc.sync.dma_start(out=x_t, in_=src)
            x_tiles.append(x_t)

        # scalar tables ride SWDGE (Pool) — the broadcast AP needs it.
        # These go first in the Pool program so sample 0 isn't delayed.
        nc.gpsimd.dma_start(out=scal_sb, in_=scal_bcast)
        nc.gpsimd.dma_start(out=rinv_sb, in_=rinv_d)

        # iota3[p, d, f] = 128 + p - 128*d - f  (so [iota3 == tx] <=>
        # [p - f == tx + 128*(d-1)]); integers <= 255, exact in fp16.
        iota3 = consts.tile([P, 3, P], F16)
        nc.gpsimd.iota(iota3, pattern=[[-128, 3], [-1, P]], base=128,
                       channel_multiplier=1, allow_small_or_imprecise_dtypes=True)
        ones_t = consts.tile([P, P], F16)
        nc.vector.memset(ones_t, 1.0)
        ones1 = consts.tile([P, 1], F16)
        nc.vector.memset(ones1, 1.0)

        # ACT func-table preload so the 1.3us LoadActFuncSet doesn't land in
        # the middle of sample 0's chain.
        warm = consts.tile([P, 1], F32)
        nc.scalar.activation(out=warm, in_=ones1, func=AF.Identity,
                             bias=0.0, scale=1.0)

        # y_pad tiles allocated once; zero borders persist across samples
        # (color stage only writes the interior columns). Both borders of a
        # tile are cleared in ONE strided memset: dims [c, kt, border, col]
        # with the border dim stepping 0 -> W+PADL.
        y_tiles = []
        for i in range(NY):
            y_t = ypads.tile([P, C, KT, WPAD], F16, tag=f"ypad{i}")
            left = y_t[:, :, :, 0:PADL]
            borders = bass.AP(
                tensor=left.tensor,
                offset=left.offset,
                ap=list(left.ap)[:-1] + [[W + PADL, 2], [1, PADL]],
            )
            nc.gpsimd.memset(borders, 0.0)
            y_tiles.append(y_t)

        pending = None  # (z_tile, s) awaiting cutout fixups + store

        def flush_pending():
            nonlocal pending
            if pending is None:
                return
            z_t, s = pending
            pending = None
            # registers for the band offsets (DVE)
            _, vals = nc.values_load_multi_w_load_instructions(
                scal_sb[0:1, s, SC_CY0:SC_CY15 + 1].bitcast(I32),
                engines=(ET.DVE,),
                min_val=0, max_val=W - BW2,
                skip_runtime_bounds_check=True,
            )
            cy0v, cy15v = vals
            for mt in range(KT):
                for cyv, bw in ((cy0v, BW1), (cy15v, BW2)):
                    nc.vector.tensor_scalar(
                        out=z_t[:, :, mt, bass.ds(cyv, bw)],
                        in0=z_t[:, :, mt, bass.ds(cyv, bw)],
                        scalar1=rinv_sb[:, s, mt:mt + 1], scalar2=None,
                        op0=OP.mult,
                    )
            z_dst = out_d[s].rearrange("c (kt p) w -> p c kt w", p=P)
            nc.sync.dma_start(out=z_dst, in_=z_t)

        for s in range(B_LOC):
            xs = x_tiles[s]  # [P, C, KT, W] f16

            # ---- S column sums straight off x (PE; free in the cost model,
            # and it runs concurrently with the mc3 adds) ----
            tpz = ptp.tile([P, 2], F32)
            chunks = [(c, kt, j) for c in range(C) for kt in range(KT)
                      for j in range(2)]
            for i, (c, kt, j) in enumerate(chunks):
                nc.tensor.matmul(
                    out=tpz[:, 0:1],
                    lhsT=xs[:, c, kt, j * P:(j + 1) * P],
                    rhs=ones1,
                    start=(i == 0), stop=(i == len(chunks) - 1),
                )

            # ---- mc3 = x0 + x1 (Pool) + x2 (DVE) ----
            # (first two samples fully on DVE: Pool is busy with startup)
            mc3a = map_.tile([P, KT, W], F16)
            if s < 2:
                nc.vector.tensor_add(mc3a, xs[:, 0], xs[:, 1])
            else:
                nc.gpsimd.tensor_add(mc3a, xs[:, 0], xs[:, 1])
            mc3 = mcp.tile([P, KT, W], F16)
            nc.vector.tensor_add(mc3, mc3a, xs[:, 2])

            sA = sap.tile([P, 1], F16)
            nc.scalar.activation(out=sA, in_=tpz[:, 0:1], func=AF.Copy,
                                 bias=0.0, scale=1.0)
            nc.tensor.matmul(out=tpz[:, 1:2], lhsT=ones_t, rhs=sA,
                             start=True, stop=True)

            # ---- D' = (Cp/As)*S + b/As (ACT), t' = (Bp/As)*mc3 + D' (DVE) ----
            D_t = dp.tile([P, 1], F32)
            nc.scalar.activation(out=D_t, in_=tpz[:, 1:2], func=AF.Identity,
                                 bias=sc(s, SC_BA), scale=sc(s, SC_CPA))
            nc.vector.tensor_scalar(
                out=mc3, in0=mc3, scalar1=sc(s, SC_AP),
                scalar2=D_t[:, 0:1], op0=OP.mult, op1=OP.add,
            )

            # ---- As-scaled shift-matrix tile (one fused DVE op) ----
            cmp_t = cmpp.tile([P, 3, P], F16)
            nc.vector.tensor_scalar(
                out=cmp_t, in0=iota3, scalar1=sc(s, SC_TX),
                scalar2=sc(s, SC_AS), op0=OP.is_equal, op1=OP.mult,
            )

            # ---- color: y_c = x_c + t', all channels in one op (t' is
            # replicated across c via a stride-0 dim) ----
            y_t = y_tiles[s % NY]
            tfull = mc3[:, :, :]
            tbc = bass.AP(
                tensor=tfull.tensor,
                offset=tfull.offset,
                ap=[list(tfull.ap[0]), [0, C]] + [list(d) for d in tfull.ap[1:]],
            )
            nc.vector.tensor_add(y_t[:, :, :, PADL:PADL + W], xs, tbc)

            # cutout fixups + store for the previous sample (keeps DVE from
            # stalling on this sample's evictions)
            flush_pending()

            # ---- H-shift matmuls: one PSUM tile per row-block, 3 channels ----
            pzm = []
            for mt in range(KT):
                pz_t = pzp.tile([P, C, 512], F32, tag="pz")
                pzm.append(pz_t)
                # kt outer so the lhsT (weights) is reused across channels
                for kt in range(KT):
                    for c in range(C):
                        nc.tensor.matmul(
                            out=pz_t[:, c, 0:WPAD],
                            lhsT=cmp_t[:, mt - kt + 1, :],
                            rhs=y_t[:, c, kt, :],
                            start=(kt == 0), stop=(kt == KT - 1),
                        )

            # ---- W-shift eviction: dynamic-start slice copy (ACT) ----
            _, vals = nc.values_load_multi_w_load_instructions(
                scal_sb[0:1, s, SC_TYOFF:SC_TYOFF + 1].bitcast(I32),
                engines=(ET.Activation,),
                min_val=0, max_val=2 * SHIFT,
                skip_runtime_bounds_check=True,
            )
            tyv = vals[0]
            z_t = zp.tile([P, C, KT, W], F16)
            for mt in range(KT):
                nc.scalar.activation(
                    out=z_t[:, :, mt, :],
                    in_=pzm[mt][:, :, bass.ds(tyv, W)],
                    func=AF.Copy, bias=0.0, scale=1.0,
                )
            pending = (z_t, s)

        flush_pending()


def host_scalars(r_bright, r_sat, r_con, t_x, t_y, off_x, off_y):
    """Per-sample scalar table [B, NSCAL] float32 (int cols bit-cast)."""
    rb = r_bright.reshape(B).astype(np.float64)
    rs = r_sat.reshape(B).astype(np.float64)
    rc = r_con.reshape(B).astype(np.float64)
    txi = t_x.reshape(B).astype(np.int64) - SHIFT   # in [-32, 32]
    tyi = t_y.reshape(B).astype(np.int64) - SHIFT
    oy = off_y.reshape(B).astype(np.int64)

    k = rc + 0.5
    s2 = 2.0 * rs
    A = k * s2
    As = np.maximum(A, A_EPS)
    Bp = k * (1.0 - s2) / 3.0
    Cp = (1.0 - k) / (3.0 * H * W)
    b = rb - 0.5
    cy0 = np.maximum(0, oy - CUT // 2)
    cy1 = np.minimum(W, oy + CUT // 2 + 1)

    tab = np.zeros((B, NSCAL), np.float32)
    tab[:, SC_TX] = txi.astype(np.float32)
    tab[:, SC_AS] = As.astype(np.float32)
    tab[:, SC_AP] = (Bp / As).astype(np.float32)
    tab[:, SC_CPA] = (Cp / As).astype(np.float32)
    tab[:, SC_BA] = (b / As).astype(np.float32)
    tab[:, SC_TYOFF] = (tyi + SHIFT).astype(np.int32).view(np.float32)
    tab[:, SC_CY0] = cy0.astype(np.int32).view(np.float32)
    tab[:, SC_CY15] = (cy1 - BW2).astype(np.int32).view(np.float32)
    return tab


def host_rinv(off_x):
    """[P, B, KT] row-indicator complement: 0 inside the cut rows, else 1."""
    ox = off_x.reshape(B).astype(np.int64)
    rx0 = np.maximum(0, ox - CUT // 2)
    rx1 = np.minimum(H, ox + CUT // 2 + 1)
    rows = (np.arange(P)[:, None, None]
            + P * np.arange(KT)[None, None, :])          # [P, 1, KT]
    inside = (rows >= rx0[None, :, None]) & (rows < rx1[None, :, None])
    return np.where(inside, 0.0, 1.0).astype(np.float32)  # [P, B, KT]


def make_in_maps(x, r_bright, r_sat, r_con, t_x, t_y, off_x, off_y):
    tab = host_scalars(r_bright, r_sat, r_con, t_x, t_y, off_x, off_y)
    rinv = host_rinv(off_x)
    x = np.ascontiguousarray(x, dtype=np.float32).astype(np.float16)
    in_maps = []
    for cid in range(N_CORES):
        lo, hi = cid * B_LOC, (cid + 1) * B_LOC
        in_maps.append({
            "x": x[lo:hi],
            "scal": tab[lo:hi],
            "rinv": np.ascontiguousarray(rinv[:, lo:hi]),
        })
    return in_maps


def kernel(x, r_bright, r_sat, r_con, t_x, t_y, off_x, off_y):
    x, r_bright, r_sat, r_con, t_x, t_y, off_x, off_y = (
        np.asarray(a) for a in (x, r_bright, r_sat, r_con, t_x, t_y, off_x, off_y)
    )
    nc = build_nc()
    in_maps = make_in_maps(x, r_bright, r_sat, r_con, t_x, t_y, off_x, off_y)
    res = bass_utils.run_bass_kernel_spmd(nc, in_maps, core_ids=list(range(N_CORES)))
    out = np.concatenate([res.results[cid]["out"] for cid in range(N_CORES)], axis=0)
    return out.astype(np.float32)


# revision 33
# speedup vs baseline: 1.0234x; 1.0234x over previous
"""DiffAugment (color jitter + translation + cutout) Trainium2 Bass kernel.

Strategy (data parallel over batch, 16 samples per core on 8 cores), fp16
end-to-end to halve DMA traffic (inputs cast fp32->fp16 during the SWDGE
load; the DRAM output tensor is fp16 and the host widens back to fp32):

  - Color math refactored so the per-pixel work is a plain tensor add:
        y_c = x_c + t',   t' = (Bp/As)*mc3 + D',  D' = (Cp/As)*S + b/As
    with As = max(A, eps). The missing factor A is folded into the H-shift
    matrix (its 0/1 entries become As via a fused is_equal*mult), so
        z_c = (As*SH) @ y_c = A*SH@x_c + SH@(Bp*mc3 + Cp*S + b)
    which matches the reference color+translate result exactly (up to the
    A->As clamp, which changes the x-term by at most eps*|x|).
  - mc3 = x0+x1 on GpSimd (stt), +x2 on DVE (tensor_add).
  - S (total sample sum) via free PE matmuls: 4 column-sum matmuls of mc3
    chunks into PSUM [128,1], evicted to SBUF, then a ones-matmul broadcast.
  - Translation: H (partition-dim) shift via the As-scaled 0/1 shift-matrix
    matmul on TensorE (built on-device from a pinned 3D iota + fused
    is_equal*mult); W (free-dim) shift via a dynamic-start slice when
    evicting PSUM (zero-bordered WPAD layout).
  - Cutout: per-sample row indicators are precomputed on host ([P, 16, 2]
    table); <=51-column band multiplies on DVE (two static-width bands whose
    union is exactly the cut range) applied one sample behind to keep DVE
    from head-of-line blocking on the ACT evictions.
  - Loads are paired (2 samples per SWDGE DMA) into one static x buffer so
    the load stream never waits on compute; stores go on the SP HWDGE ring.
"""

import sys

if "/opt/trn_rl_repo" not in sys.path:
    sys.path.insert(0, "/opt/trn_rl_repo")

import numpy as np

import concourse.bass as bass
import concourse.bacc as bacc
import concourse.tile as tile
import concourse.mybir as mybir
from concourse import bass_utils

F32 = mybir.dt.float32
F16 = mybir.dt.float16
I32 = mybir.dt.int32
AF = mybir.ActivationFunctionType
OP = mybir.AluOpType
ET = mybir.EngineType

N_CORES = 8
B = 128
B_LOC = B // N_CORES  # 16
C, H, W = 3, 256, 256
KT = 2          # number of 128-row partition tiles per image
P = 128
WPAD = W + 64   # W-padded free dim (32 zero cols each side)
PADL = 32
SHIFT = 32      # int(H * 0.125 + 0.5)
CUT = 51        # int(H * 0.2 + 0.5)
BW1, BW2 = 25, 26   # static fixup band widths (union covers any 26..51 range)
A_EPS = 1e-3

# scalar table columns (f32 block, then int32-bit-cast block)
(SC_TX, SC_AS, SC_AP, SC_CPA, SC_BA,
 SC_TYOFF, SC_CY0, SC_CY15) = range(8)
NSCAL = 8

_CACHE = {}


def build_nc():
    """Build + compile the per-core Bass program (cached)."""
    if "nc" in _CACHE:
        return _CACHE["nc"]

    nc = bacc.Bacc(
        "TRN2",
        target_bir_lowering=False,
        debug=False,
        enable_asserts=True,
        num_devices=N_CORES,
    )
    x_d = nc.dram_tensor("x", [B_LOC, C, H, W], F16, kind="ExternalInput").ap()
    scal_d = nc.dram_tensor("scal", [B_LOC, NSCAL], F32, kind="ExternalInput").ap()
    rinv_d = nc.dram_tensor("rinv", [P, B_LOC, KT], F32, kind="ExternalInput").ap()
    out_d = nc.dram_tensor("out", [B_LOC, C, H, W], F16, kind="ExternalOutput").ap()

    with tile.TileContext(nc) as tc:
        _kernel_body(tc, nc, x_d, scal_d, rinv_d, out_d)

    nc.compile()
    _CACHE["nc"] = nc
    return nc


def _kernel_body(tc, nc, x_d, scal_d, rinv_d, out_d):
    NY = 6  # y_pad rotation depth (borders memset once)

    with (
        tc.tile_pool(name="consts", bufs=1) as consts,
        tc.tile_pool(name="ypads", bufs=NY) as ypads,
        tc.tile_pool(name="xt", bufs=10) as xp,
        tc.tile_pool(name="mc3a", bufs=6) as map_,
        tc.tile_pool(name="mc3", bufs=6) as mcp,
        tc.tile_pool(name="cmp", bufs=6) as cmpp,
        tc.tile_pool(name="zt", bufs=6) as zp,
        tc.tile_pool(name="sA", bufs=6) as sap,
        tc.tile_pool(name="Dt", bufs=6) as dp,
        tc.tile_pool(name="pz", bufs=2, space="PSUM") as pzp,
        tc.tile_pool(name="ptiny", bufs=2, space="PSUM") as ptp,
    ):
        # ---- constants ----
        scal_sb = consts.tile([P, B_LOC, NSCAL], F32)
        scal_bcast = bass.AP(
            tensor=scal_d.tensor,
            offset=scal_d.offset,
            ap=[[0, P]] + list(scal_d.ap),
        )
        rinv_sb = consts.tile([P, B_LOC, KT], F32)

        def sc(s, col):  # [128,1] per-sample scalar broadcast column
            return scal_sb[:, s, col:col + 1]

        # The host passes x already in fp16, so loads ride the HWDGE sync
        # ring (SP) and are all issued up front; the pool sem keeps the
        # stream well ahead of compute.
        x_tiles = []
        for s in range(B_LOC):
            x_t = xp.tile([P, C, KT, W], F16)
            src = x_d[s].rearrange("c (kt p) w -> p c kt w", p=P)
            nc.sync.dma_start(out=x_t, in_=src)
            x_tiles.append(x_t)

        # scalar tables ride SWDGE (Pool) — the broadcast AP needs it.
        # These go first in the Pool program so sample 0 isn't delayed.
        nc.gpsimd.dma_start(out=scal_sb, in_=scal_bcast)
        nc.gpsimd.dma_start(out=rinv_sb, in_=rinv_d)

        # iota3[p, d, f] = 128 + p - 128*d - f  (so [iota3 == tx] <=>
        # [p - f == tx + 128*(d-1)]); integers <= 255, exact in fp16.
        iota3 = consts.tile([P, 3, P], F16)
        nc.gpsimd.iota(iota3, pattern=[[-128, 3], [-1, P]], base=128,
                       channel_multiplier=1, allow_small_or_imprecise_dtypes=True)
        ones_t = consts.tile([P, P], F16)
        nc.vector.memset(ones_t, 1.0)
        ones1 = consts.tile([P, 1], F16)
        nc.vector.memset(ones1, 1.0)

        # ACT func-table preload so the 1.3us LoadActFuncSet doesn't land in
        # the middle of sample 0's chain.
        warm = consts.tile([P, 1], F32)
        nc.scalar.activation(out=warm, in_=ones1, func=AF.Identity,
                             bias=0.0, scale=1.0)

        # y_pad tiles allocated once; zero borders persist across samples
        # (color stage only writes the interior columns). Both borders of a
        # tile are cleared in ONE strided memset: dims [c, kt, border, col]
        # with the border dim stepping 0 -> W+PADL.
        y_tiles = []
        for i in range(NY):
            y_t = ypads.tile([P, C, KT, WPAD], F16, tag=f"ypad{i}")
            left = y_t[:, :, :, 0:PADL]
            borders = bass.AP(
                tensor=left.tensor,
                offset=left.offset,
                ap=list(left.ap)[:-1] + [[W + PADL, 2], [1, PADL]],
            )
            nc.gpsimd.memset(borders, 0.0)
            y_tiles.append(y_t)

        pending = None  # (z_tile, s) awaiting cutout fixups + store

        def flush_pending():
            nonlocal pending
            if pending is None:
                return
            z_t, s = pending
            pending = None
            # registers for the band offsets (DVE)
            _, vals = nc.values_load_multi_w_load_instructions(
                scal_sb[0:1, s, SC_CY0:SC_CY15 + 1].bitcast(I32),
                engines=(ET.DVE,),
                min_val=0, max_val=W - BW2,
                skip_runtime_bounds_check=True,
            )
            cy0v, cy15v = vals
            for mt in range(KT):
                for cyv, bw in ((cy0v, BW1), (cy15v, BW2)):
                    nc.vector.tensor_scalar(
                        out=z_t[:, :, mt, bass.ds(cyv, bw)],
                        in0=z_t[:, :, mt, bass.ds(cyv, bw)],
                        scalar1=rinv_sb[:, s, mt:mt + 1], scalar2=None,
                        op0=OP.mult,
                    )
            z_dst = out_d[s].rearrange("c (kt p) w -> p c kt w", p=P)
            nc.sync.dma_start(out=z_dst, in_=z_t)

        for s in range(B_LOC):
            xs = x_tiles[s]  # [P, C, KT, W] f16

            # ---- S column sums straight off x (PE; free in the cost model,
            # and it runs concurrently with the mc3 adds) ----
            tpz = ptp.tile([P, 2], F32)
            chunks = [(c, kt, j) for c in range(C) for kt in range(KT)
                      for j in range(2)]
            for i, (c, kt, j) in enumerate(chunks):
                nc.tensor.matmul(
                    out=tpz[:, 0:1],
                    lhsT=xs[:, c, kt, j * P:(j + 1) * P],
                    rhs=ones1,
                    start=(i == 0), stop=(i == len(chunks) - 1),
                )

            # ---- mc3 = x0 + x1 (Pool) + x2 (DVE) ----
            # (first two samples fully on DVE: Pool is busy with startup)
            mc3a = map_.tile([P, KT, W], F16)
            if s < 2:
                nc.vector.tensor_add(mc3a, xs[:, 0], xs[:, 1])
            else:
                nc.gpsimd.tensor_add(mc3a, xs[:, 0], xs[:, 1])
            mc3 = mcp.tile([P, KT, W], F16)
            nc.vector.tensor_add(mc3, mc3a, xs[:, 2])

            sA = sap.tile([P, 1], F16)
            nc.scalar.activation(out=sA, in_=tpz[:, 0:1], func=AF.Copy,
                                 bias=0.0, scale=1.0)
            nc.tensor.matmul(out=tpz[:, 1:2], lhsT=ones_t, rhs=sA,
                             start=True, stop=True)

            # ---- D' = (Cp/As)*S + b/As (ACT), t' = (Bp/As)*mc3 + D' (DVE) ----
            D_t = dp.tile([P, 1], F32)
            nc.scalar.activation(out=D_t, in_=tpz[:, 1:2], func=AF.Identity,
                                 bias=sc(s, SC_BA), scale=sc(s, SC_CPA))
            nc.vector.tensor_scalar(
                out=mc3, in0=mc3, scalar1=sc(s, SC_AP),
                scalar2=D_t[:, 0:1], op0=OP.mult, op1=OP.add,
            )

            # ---- As-scaled shift-matrix tile (one fused DVE op) ----
            cmp_t = cmpp.tile([P, 3, P], F16)
            nc.vector.tensor_scalar(
                out=cmp_t, in0=iota3, scalar1=sc(s, SC_TX),
                scalar2=sc(s, SC_AS), op0=OP.is_equal, op1=OP.mult,
            )

            # ---- color: y_c = x_c + t' ----
            y_t = y_tiles[s % NY]
            for c in range(C):
                nc.vector.tensor_add(y_t[:, c, :, PADL:PADL + W], xs[:, c], mc3)

            # cutout fixups + store for the previous sample (keeps DVE from
            # stalling on this sample's evictions)
            flush_pending()

            # ---- H-shift matmuls: one PSUM tile per row-block, 3 channels ----
            pzm = []
            for mt in range(KT):
                pz_t = pzp.tile([P, C, 512], F32, tag="pz")
                pzm.append(pz_t)
                # kt outer so the lhsT (weights) is reused across channels
                for kt in range(KT):
                    for c in range(C):
                        nc.tensor.matmul(
                            out=pz_t[:, c, 0:WPAD],
                            lhsT=cmp_t[:, mt - kt + 1, :],
                            rhs=y_t[:, c, kt, :],
                            start=(kt == 0), stop=(kt == KT - 1),
                        )

            # ---- W-shift eviction: dynamic-start slice copy (ACT) ----
            _, vals = nc.values_load_multi_w_load_instructions(
                scal_sb[0:1, s, SC_TYOFF:SC_TYOFF + 1].bitcast(I32),
                engines=(ET.Activation,),
                min_val=0, max_val=2 * SHIFT,
                skip_runtime_bounds_check=True,
            )
            tyv = vals[0]
            z_t = zp.tile([P, C, KT, W], F16)
            for mt in range(KT):
                nc.scalar.activation(
                    out=z_t[:, :, mt, :],
                    in_=pzm[mt][:, :, bass.ds(tyv, W)],
                    func=AF.Copy, bias=0.0, scale=1.0,
                )
            pending = (z_t, s)

        flush_pending()


def host_scalars(r_bright, r_sat, r_con, t_x, t_y, off_x, off_y):
    """Per-sample scalar table [B, NSCAL] float32 (int cols bit-cast)."""
    rb = r_bright.reshape(B).astype(np.float64)
    rs = r_sat.reshape(B).astype(np.float64)
    rc = r_con.reshape(B).astype(np.float64)
    txi = t_x.reshape(B).astype(np.int64) - SHIFT   # in [-32, 32]
    tyi = t_y.reshape(B).astype(np.int64) - SHIFT
    oy = off_y.reshape(B).astype(np.int64)

    k = rc + 0.5
    s2 = 2.0 * rs
    A = k * s2
    As = np.maximum(A, A_EPS)
    Bp = k * (1.0 - s2) / 3.0
    Cp = (1.0 - k) / (3.0 * H * W)
    b = rb - 0.5
    cy0 = np.maximum(0, oy - CUT // 2)
    cy1 = np.minimum(W, oy + CUT // 2 + 1)

    tab = np.zeros((B, NSCAL), np.float32)
    tab[:, SC_TX] = txi.astype(np.float32)
    tab[:, SC_AS] = As.astype(np.float32)
    tab[:, SC_AP] = (Bp / As).astype(np.float32)
    tab[:, SC_CPA] = (Cp / As).astype(np.float32)
    tab[:, SC_BA] = (b / As).astype(np.float32)
    tab[:, SC_TYOFF] = (tyi + SHIFT).astype(np.int32).view(np.float32)
    tab[:, SC_CY0] = cy0.astype(np.int32).view(np.float32)
    tab[:, SC_CY15] = (cy1 - BW2).astype(np.int32).view(np.float32)
    return tab


def host_rinv(off_x):
    """[P, B, KT] row-indicator complement: 0 inside the cut rows, else 1."""
    ox = off_x.reshape(B).astype(np.int64)
    rx0 = np.maximum(0, ox - CUT // 2)
    rx1 = np.minimum(H, ox + CUT // 2 + 1)
    rows = (np.arange(P)[:, None, None]
            + P * np.arange(KT)[None, None, :])          # [P, 1, KT]
    inside = (rows >= rx0[None, :, None]) & (rows < rx1[None, :, None])
    return np.where(inside, 0.0, 1.0).astype(np.float32)  # [P, B, KT]


def make_in_maps(x, r_bright, r_sat, r_con, t_x, t_y, off_x, off_y):
    tab = host_scalars(r_bright, r_sat, r_con, t_x, t_y, off_x, off_y)
    rinv = host_rinv(off_x)
    x = np.ascontiguousarray(x, dtype=np.float32).astype(np.float16)
    in_maps = []
    for cid in range(N_CORES):
        lo, hi = cid * B_LOC, (cid + 1) * B_LOC
        in_maps.append({
            "x": x[lo:hi],
            "scal": tab[lo:hi],
            "rinv": np.ascontiguousarray(rinv[:, lo:hi]),
        })
    return in_maps


def kernel(x, r_bright, r_sat, r_con, t_x, t_y, off_x, off_y):
    x, r_bright, r_sat, r_con, t_x, t_y, off_x, off_y = (
        np.asarray(a) for a in (x, r_bright, r_sat, r_con, t_x, t_y, off_x, off_y)
    )
    nc = build_nc()
    in_maps = make_in_maps(x, r_bright, r_sat, r_con, t_x, t_y, off_x, off_y)
    res = bass_utils.run_bass_kernel_spmd(nc, in_maps, core_ids=list(range(N_CORES)))
    out = np.concatenate([res.results[cid]["out"] for cid in range(N_CORES)], axis=0)
    return out.astype(np.float32)


# revision 40
# speedup vs baseline: 1.0430x; 1.0192x over previous
"""DiffAugment (color jitter + translation + cutout) Trainium2 Bass kernel.

Strategy (data parallel over batch, 16 samples per core on 8 cores), fp16
end-to-end to halve DMA traffic (inputs cast fp32->fp16 during the SWDGE
load; the DRAM output tensor is fp16 and the host widens back to fp32):

  - Color math refactored so the per-pixel work is a plain tensor add:
        y_c = x_c + t',   t' = (Bp/As)*mc3 + D',  D' = (Cp/As)*S + b/As
    with As = max(A, eps). The missing factor A is folded into the H-shift
    matrix (its 0/1 entries become As via a fused is_equal*mult), so
        z_c = (As*SH) @ y_c = A*SH@x_c + SH@(Bp*mc3 + Cp*S + b)
    which matches the reference color+translate result exactly (up to the
    A->As clamp, which changes the x-term by at most eps*|x|).
  - mc3 = x0+x1 on GpSimd (stt), +x2 on DVE (tensor_add).
  - S (total sample sum) via free PE matmuls: 4 column-sum matmuls of mc3
    chunks into PSUM [128,1], evicted to SBUF, then a ones-matmul broadcast.
  - Translation: H (partition-dim) shift via the As-scaled 0/1 shift-matrix
    matmul on TensorE (built on-device from a pinned 3D iota + fused
    is_equal*mult); W (free-dim) shift via a dynamic-start slice when
    evicting PSUM (zero-bordered WPAD layout).
  - Cutout: per-sample row indicators are precomputed on host ([P, 16, 2]
    table); <=51-column band multiplies on DVE (two static-width bands whose
    union is exactly the cut range) applied one sample behind to keep DVE
    from head-of-line blocking on the ACT evictions.
  - Loads are paired (2 samples per SWDGE DMA) into one static x buffer so
    the load stream never waits on compute; stores go on the SP HWDGE ring.
"""

import sys

if "/opt/trn_rl_repo" not in sys.path:
    sys.path.insert(0, "/opt/trn_rl_repo")

import numpy as np

import concourse.bass as bass
import concourse.bacc as bacc
import concourse.tile as tile
import concourse.mybir as mybir
from concourse import bass_utils

F32 = mybir.dt.float32
F16 = mybir.dt.float16
I32 = mybir.dt.int32
AF = mybir.ActivationFunctionType
OP = mybir.AluOpType
ET = mybir.EngineType

N_CORES = 8
B = 128
B_LOC = B // N_CORES  # 16
C, H, W = 3, 256, 256
KT = 2          # number of 128-row partition tiles per image
P = 128
WPAD = W + 64   # W-padded free dim (32 zero cols each side)
PADL = 32
SHIFT = 32      # int(H * 0.125 + 0.5)
CUT = 51        # int(H * 0.2 + 0.5)
BW1, BW2 = 25, 26   # static fixup band widths (union covers any 26..51 range)
A_EPS = 1e-3

# scalar table columns (f32 block, then int32-bit-cast block)
(SC_TX, SC_AS, SC_AP, SC_CPA, SC_BA,
 SC_TYOFF, SC_CY0, SC_CY15) = range(8)
NSCAL = 8

_CACHE = {}


def build_nc():
    """Build + compile the per-core Bass program (cached)."""
    if "nc" in _CACHE:
        return _CACHE["nc"]

    nc = bacc.Bacc(
        "TRN2",
        target_bir_lowering=False,
        debug=False,
        enable_asserts=True,
        num_devices=N_CORES,
    )
    x_d = nc.dram_tensor("x", [B_LOC, C, H, W], F16, kind="ExternalInput").ap()
    scal_d = nc.dram_tensor("scal", [B_LOC, NSCAL], F32, kind="ExternalInput").ap()
    rinv_d = nc.dram_tensor("rinv", [P, B_LOC, KT], F32, kind="ExternalInput").ap()
    out_d = nc.dram_tensor("out", [B_LOC, C, H, W], F16, kind="ExternalOutput").ap()

    with tile.TileContext(nc) as tc:
        _kernel_body(tc, nc, x_d, scal_d, rinv_d, out_d)

    nc.compile()
    _CACHE["nc"] = nc
    return nc


def _kernel_body(tc, nc, x_d, scal_d, rinv_d, out_d):
    NY = 6  # y_pad rotation depth (borders memset once)

    with (
        tc.tile_pool(name="consts", bufs=1) as consts,
        tc.tile_pool(name="ypads", bufs=NY) as ypads,
        tc.tile_pool(name="xt", bufs=10) as xp,
        tc.tile_pool(name="mc3a", bufs=6) as map_,
        tc.tile_pool(name="mc3", bufs=6) as mcp,
        tc.tile_pool(name="cmp", bufs=6) as cmpp,
        tc.tile_pool(name="zt", bufs=6) as zp,
        tc.tile_pool(name="sA", bufs=6) as sap,
        tc.tile_pool(name="Dt", bufs=6) as dp,
        tc.tile_pool(name="pz", bufs=2, space="PSUM") as pzp,
        tc.tile_pool(name="ptiny", bufs=2, space="PSUM") as ptp,
    ):
        # ---- constants ----
        scal_sb = consts.tile([P, B_LOC, NSCAL], F32)
        scal_bcast = bass.AP(
            tensor=scal_d.tensor,
            offset=scal_d.offset,
            ap=[[0, P]] + list(scal_d.ap),
        )
        rinv_sb = consts.tile([P, B_LOC, KT], F32)

        def sc(s, col):  # [128,1] per-sample scalar broadcast column
            return scal_sb[:, s, col:col + 1]

        # The host passes x already in fp16, so loads ride the HWDGE sync
        # ring (SP) and are all issued up front; the pool sem keeps the
        # stream well ahead of compute.
        x_tiles = []
        for s in range(B_LOC):
            x_t = xp.tile([P, C, KT, W], F16)
            src = x_d[s].rearrange("c (kt p) w -> p c kt w", p=P)
            nc.sync.dma_start(out=x_t, in_=src)
            x_tiles.append(x_t)

        # scalar tables ride SWDGE (Pool) — the broadcast AP needs it.
        # These go first in the Pool program so sample 0 isn't delayed.
        nc.gpsimd.dma_start(out=scal_sb, in_=scal_bcast)
        nc.gpsimd.dma_start(out=rinv_sb, in_=rinv_d)

        # iota3[p, d, f] = 128 + p - 128*d - f  (so [iota3 == tx] <=>
        # [p - f == tx + 128*(d-1)]); integers <= 255, exact in fp16.
        iota3 = consts.tile([P, 3, P], F16)
        nc.gpsimd.iota(iota3, pattern=[[-128, 3], [-1, P]], base=128,
                       channel_multiplier=1, allow_small_or_imprecise_dtypes=True)
        ones_t = consts.tile([P, P], F16)
        nc.vector.memset(ones_t, 1.0)
        ones1 = consts.tile([P, 1], F16)
        nc.vector.memset(ones1, 1.0)

        # ACT func-table preload so the 1.3us LoadActFuncSet doesn't land in
        # the middle of sample 0's chain.
        warm = consts.tile([P, 1], F32)
        nc.scalar.activation(out=warm, in_=ones1, func=AF.Identity,
                             bias=0.0, scale=1.0)

        # y_pad tiles allocated once; zero borders persist across samples
        # (color stage only writes the interior columns). Both borders of a
        # tile are cleared in ONE strided memset: dims [c, kt, border, col]
        # with the border dim stepping 0 -> W+PADL.
        y_tiles = []
        for i in range(NY):
            y_t = ypads.tile([P, C, KT, WPAD], F16, tag=f"ypad{i}")
            left = y_t[:, :, :, 0:PADL]
            borders = bass.AP(
                tensor=left.tensor,
                offset=left.offset,
                ap=list(left.ap)[:-1] + [[W + PADL, 2], [1, PADL]],
            )
            nc.gpsimd.memset(borders, 0.0)
            y_tiles.append(y_t)

        # one PE register per sample for the W-shift window (loaded up
        # front so registers are never reused/clobbered across samples)
        _, tyvals = nc.values_load_multi_w_load_instructions(
            scal_sb[0:1, :, SC_TYOFF:SC_TYOFF + 1].bitcast(I32),
            engines=(ET.PE,),
            min_val=0, max_val=2 * SHIFT,
            skip_runtime_bounds_check=True,
        )

        pending = None  # (z_tile, s) awaiting cutout fixups + store

        def flush_pending():
            nonlocal pending
            if pending is None:
                return
            z_t, s = pending
            pending = None
            # registers for the band offsets (DVE)
            _, vals = nc.values_load_multi_w_load_instructions(
                scal_sb[0:1, s, SC_CY0:SC_CY15 + 1].bitcast(I32),
                engines=(ET.DVE,),
                min_val=0, max_val=W - BW2,
                skip_runtime_bounds_check=True,
            )
            cy0v, cy15v = vals
            for mt in range(KT):
                for cyv, bw in ((cy0v, BW1), (cy15v, BW2)):
                    nc.vector.tensor_scalar(
                        out=z_t[:, :, mt, bass.ds(cyv, bw)],
                        in0=z_t[:, :, mt, bass.ds(cyv, bw)],
                        scalar1=rinv_sb[:, s, mt:mt + 1], scalar2=None,
                        op0=OP.mult,
                    )
            z_dst = out_d[s].rearrange("c (kt p) w -> p c kt w", p=P)
            nc.sync.dma_start(out=z_dst, in_=z_t)

        for s in range(B_LOC):
            xs = x_tiles[s]  # [P, C, KT, W] f16

            # ---- S column sums straight off x (PE; free in the cost model,
            # and it runs concurrently with the mc3 adds) ----
            tpz = ptp.tile([P, 2], F32)
            chunks = [(c, kt, j) for c in range(C) for kt in range(KT)
                      for j in range(2)]
            for i, (c, kt, j) in enumerate(chunks):
                nc.tensor.matmul(
                    out=tpz[:, 0:1],
                    lhsT=xs[:, c, kt, j * P:(j + 1) * P],
                    rhs=ones1,
                    start=(i == 0), stop=(i == len(chunks) - 1),
                )

            # ---- mc3 = x0 + x1 (Pool) + x2 (DVE) ----
            # (first two samples fully on DVE: Pool is busy with startup)
            mc3a = map_.tile([P, KT, W], F16)
            if s < 2:
                nc.vector.tensor_add(mc3a, xs[:, 0], xs[:, 1])
            else:
                nc.gpsimd.tensor_add(mc3a, xs[:, 0], xs[:, 1])
            mc3 = mcp.tile([P, KT, W], F16)
            nc.vector.tensor_add(mc3, mc3a, xs[:, 2])

            sA = sap.tile([P, 1], F16)
            nc.scalar.activation(out=sA, in_=tpz[:, 0:1], func=AF.Copy,
                                 bias=0.0, scale=1.0)
            nc.tensor.matmul(out=tpz[:, 1:2], lhsT=ones_t, rhs=sA,
                             start=True, stop=True)

            # ---- D' = (Cp/As)*S + b/As (ACT), t' = (Bp/As)*mc3 + D' (DVE) ----
            D_t = dp.tile([P, 1], F32)
            nc.scalar.activation(out=D_t, in_=tpz[:, 1:2], func=AF.Identity,
                                 bias=sc(s, SC_BA), scale=sc(s, SC_CPA))
            nc.vector.tensor_scalar(
                out=mc3, in0=mc3, scalar1=sc(s, SC_AP),
                scalar2=D_t[:, 0:1], op0=OP.mult, op1=OP.add,
            )

            # ---- As-scaled shift-matrix tile (one fused DVE op) ----
            cmp_t = cmpp.tile([P, 3, P], F16)
            nc.vector.tensor_scalar(
                out=cmp_t, in0=iota3, scalar1=sc(s, SC_TX),
                scalar2=sc(s, SC_AS), op0=OP.is_equal, op1=OP.mult,
            )

            # ---- color: y_c = x_c + t' ----
            y_t = y_tiles[s % NY]
            for c in range(C):
                nc.vector.tensor_add(y_t[:, c, :, PADL:PADL + W], xs[:, c], mc3)

            # cutout fixups + store for the previous sample (keeps DVE from
            # stalling on this sample's evictions)
            flush_pending()

            # ---- H-shift matmuls (W-shift folded in: the rhs reads the
            # dynamic 256-wide window of the padded y, so psum tiles are
            # half as wide and three of them fit in PSUM) ----
            tyv = tyvals[s]
            pzm = []
            for mt in range(KT):
                pz_t = pzp.tile([P, C, W], F32, tag="pz")
                pzm.append(pz_t)
                # c outer: each psum accumulation chain stays contiguous
                # (interleaved start/stop groups miscompile)
                for c in range(C):
                    for kt in range(KT):
                        nc.tensor.matmul(
                            out=pz_t[:, c, :],
                            lhsT=cmp_t[:, mt - kt + 1, :],
                            rhs=y_t[:, c, kt, bass.ds(tyv, W)],
                            start=(kt == 0), stop=(kt == KT - 1),
                        )

            # ---- eviction: plain copy (ACT) ----
            z_t = zp.tile([P, C, KT, W], F16)
            for mt in range(KT):
                nc.scalar.activation(
                    out=z_t[:, :, mt, :],
                    in_=pzm[mt][:, :, :],
                    func=AF.Copy, bias=0.0, scale=1.0,
                )
            pending = (z_t, s)

        flush_pending()


def host_scalars(r_bright, r_sat, r_con, t_x, t_y, off_x, off_y):
    """Per-sample scalar table [B, NSCAL] float32 (int cols bit-cast)."""
    rb = r_bright.reshape(B).astype(np.float64)
    rs = r_sat.reshape(B).astype(np.float64)
    rc = r_con.reshape(B).astype(np.float64)
    txi = t_x.reshape(B).astype(np.int64) - SHIFT   # in [-32, 32]
    tyi = t_y.reshape(B).astype(np.int64) - SHIFT
    oy = off_y.reshape(B).astype(np.int64)

    k = rc + 0.5
    s2 = 2.0 * rs
    A = k * s2
    As = np.maximum(A, A_EPS)
    Bp = k * (1.0 - s2) / 3.0
    Cp = (1.0 - k) / (3.0 * H * W)
    b = rb - 0.5
    cy0 = np.maximum(0, oy - CUT // 2)
    cy1 = np.minimum(W, oy + CUT // 2 + 1)

    tab = np.zeros((B, NSCAL), np.float32)
    tab[:, SC_TX] = txi.astype(np.float32)
    tab[:, SC_AS] = As.astype(np.float32)
    tab[:, SC_AP] = (Bp / As).astype(np.float32)
    tab[:, SC_CPA] = (Cp / As).astype(np.float32)
    tab[:, SC_BA] = (b / As).astype(np.float32)
    tab[:, SC_TYOFF] = (tyi + SHIFT).astype(np.int32).view(np.float32)
    tab[:, SC_CY0] = cy0.astype(np.int32).view(np.float32)
    tab[:, SC_CY15] = (cy1 - BW2).astype(np.int32).view(np.float32)
    return tab


def host_rinv(off_x):
    """[P, B, KT] row-indicator complement: 0 inside the cut rows, else 1."""
    ox = off_x.reshape(B).astype(np.int64)
    rx0 = np.maximum(0, ox - CUT // 2)
    rx1 = np.minimum(H, ox + CUT // 2 + 1)
    rows = (np.arange(P)[:, None, None]
            + P * np.arange(KT)[None, None, :])          # [P, 1, KT]
    inside = (rows >= rx0[None, :, None]) & (rows < rx1[None, :, None])
    return np.where(inside, 0.0, 1.0).astype(np.float32)  # [P, B, KT]


def make_in_maps(x, r_bright, r_sat, r_con, t_x, t_y, off_x, off_y):
    tab = host_scalars(r_bright, r_sat, r_con, t_x, t_y, off_x, off_y)
    rinv = host_rinv(off_x)
    x = np.ascontiguousarray(x, dtype=np.float32).astype(np.float16)
    in_maps = []
    for cid in range(N_CORES):
        lo, hi = cid * B_LOC, (cid + 1) * B_LOC
        in_maps.append({
            "x": x[lo:hi],
            "scal": tab[lo:hi],
            "rinv": np.ascontiguousarray(rinv[:, lo:hi]),
        })
    return in_maps


def kernel(x, r_bright, r_sat, r_con, t_x, t_y, off_x, off_y):
    x, r_bright, r_sat, r_con, t_x, t_y, off_x, off_y = (
        np.asarray(a) for a in (x, r_bright, r_sat, r_con, t_x, t_y, off_x, off_y)
    )
    nc = build_nc()
    in_maps = make_in_maps(x, r_bright, r_sat, r_con, t_x, t_y, off_x, off_y)
    res = bass_utils.run_bass_kernel_spmd(nc, in_maps, core_ids=list(range(N_CORES)))
    out = np.concatenate([res.results[cid]["out"] for cid in range(N_CORES)], axis=0)
    return out.astype(np.float32)


# revision 41
# speedup vs baseline: 1.0733x; 1.0290x over previous
"""DiffAugment (color jitter + translation + cutout) Trainium2 Bass kernel.

Strategy (data parallel over batch, 16 samples per core on 8 cores), fp16
end-to-end to halve DMA traffic (inputs cast fp32->fp16 during the SWDGE
load; the DRAM output tensor is fp16 and the host widens back to fp32):

  - Color math refactored so the per-pixel work is a plain tensor add:
        y_c = x_c + t',   t' = (Bp/As)*mc3 + D',  D' = (Cp/As)*S + b/As
    with As = max(A, eps). The missing factor A is folded into the H-shift
    matrix (its 0/1 entries become As via a fused is_equal*mult), so
        z_c = (As*SH) @ y_c = A*SH@x_c + SH@(Bp*mc3 + Cp*S + b)
    which matches the reference color+translate result exactly (up to the
    A->As clamp, which changes the x-term by at most eps*|x|).
  - mc3 = x0+x1 on GpSimd (stt), +x2 on DVE (tensor_add).
  - S (total sample sum) via free PE matmuls: 4 column-sum matmuls of mc3
    chunks into PSUM [128,1], evicted to SBUF, then a ones-matmul broadcast.
  - Translation: H (partition-dim) shift via the As-scaled 0/1 shift-matrix
    matmul on TensorE (built on-device from a pinned 3D iota + fused
    is_equal*mult); W (free-dim) shift via a dynamic-start slice when
    evicting PSUM (zero-bordered WPAD layout).
  - Cutout: per-sample row indicators are precomputed on host ([P, 16, 2]
    table); <=51-column band multiplies on DVE (two static-width bands whose
    union is exactly the cut range) applied one sample behind to keep DVE
    from head-of-line blocking on the ACT evictions.
  - Loads are paired (2 samples per SWDGE DMA) into one static x buffer so
    the load stream never waits on compute; stores go on the SP HWDGE ring.
"""

import sys

if "/opt/trn_rl_repo" not in sys.path:
    sys.path.insert(0, "/opt/trn_rl_repo")

import numpy as np

import concourse.bass as bass
import concourse.bacc as bacc
import concourse.tile as tile
import concourse.mybir as mybir
from concourse import bass_utils

F32 = mybir.dt.float32
F16 = mybir.dt.float16
I32 = mybir.dt.int32
AF = mybir.ActivationFunctionType
OP = mybir.AluOpType
ET = mybir.EngineType

N_CORES = 8
B = 128
B_LOC = B // N_CORES  # 16
C, H, W = 3, 256, 256
KT = 2          # number of 128-row partition tiles per image
P = 128
WPAD = W + 64   # W-padded free dim (32 zero cols each side)
PADL = 32
SHIFT = 32      # int(H * 0.125 + 0.5)
CUT = 51        # int(H * 0.2 + 0.5)
BW1, BW2 = 25, 26   # static fixup band widths (union covers any 26..51 range)
A_EPS = 1e-3

# scalar table columns (f32 block, then int32-bit-cast block)
(SC_TX, SC_AS, SC_AP, SC_CPA, SC_BA,
 SC_TYOFF, SC_CY0, SC_CY15) = range(8)
NSCAL = 8

_CACHE = {}


def build_nc():
    """Build + compile the per-core Bass program (cached)."""
    if "nc" in _CACHE:
        return _CACHE["nc"]

    nc = bacc.Bacc(
        "TRN2",
        target_bir_lowering=False,
        debug=False,
        enable_asserts=True,
        num_devices=N_CORES,
    )
    x_d = nc.dram_tensor("x", [B_LOC, C, H, W], F16, kind="ExternalInput").ap()
    scal_d = nc.dram_tensor("scal", [B_LOC, NSCAL], F32, kind="ExternalInput").ap()
    rinv_d = nc.dram_tensor("rinv", [P, B_LOC, KT], F32, kind="ExternalInput").ap()
    out_d = nc.dram_tensor("out", [B_LOC, C, H, W], F16, kind="ExternalOutput").ap()

    with tile.TileContext(nc) as tc:
        _kernel_body(tc, nc, x_d, scal_d, rinv_d, out_d)

    nc.compile()
    _CACHE["nc"] = nc
    return nc


def _kernel_body(tc, nc, x_d, scal_d, rinv_d, out_d):
    NY = 6  # y_pad rotation depth (borders memset once)

    with (
        tc.tile_pool(name="consts", bufs=1) as consts,
        tc.tile_pool(name="ypads", bufs=NY) as ypads,
        tc.tile_pool(name="xt", bufs=10) as xp,
        tc.tile_pool(name="mc3a", bufs=6) as map_,
        tc.tile_pool(name="mc3", bufs=6) as mcp,
        tc.tile_pool(name="cmp", bufs=6) as cmpp,
        tc.tile_pool(name="zt", bufs=6) as zp,
        tc.tile_pool(name="sA", bufs=6) as sap,
        tc.tile_pool(name="Dt", bufs=6) as dp,
        tc.tile_pool(name="pz", bufs=3, space="PSUM") as pzp,
        tc.tile_pool(name="ptiny", bufs=2, space="PSUM") as ptp,
    ):
        # ---- constants ----
        scal_sb = consts.tile([P, B_LOC, NSCAL], F32)
        scal_bcast = bass.AP(
            tensor=scal_d.tensor,
            offset=scal_d.offset,
            ap=[[0, P]] + list(scal_d.ap),
        )
        rinv_sb = consts.tile([P, B_LOC, KT], F32)

        def sc(s, col):  # [128,1] per-sample scalar broadcast column
            return scal_sb[:, s, col:col + 1]

        # The host passes x already in fp16, so loads ride the HWDGE sync
        # ring (SP) and are all issued up front; the pool sem keeps the
        # stream well ahead of compute.
        x_tiles = []
        for s in range(B_LOC):
            x_t = xp.tile([P, C, KT, W], F16)
            src = x_d[s].rearrange("c (kt p) w -> p c kt w", p=P)
            nc.sync.dma_start(out=x_t, in_=src)
            x_tiles.append(x_t)

        # scalar tables ride SWDGE (Pool) — the broadcast AP needs it.
        # These go first in the Pool program so sample 0 isn't delayed.
        nc.gpsimd.dma_start(out=scal_sb, in_=scal_bcast)
        nc.gpsimd.dma_start(out=rinv_sb, in_=rinv_d)

        # iota3[p, d, f] = 128 + p - 128*d - f  (so [iota3 == tx] <=>
        # [p - f == tx + 128*(d-1)]); integers <= 255, exact in fp16.
        iota3 = consts.tile([P, 3, P], F16)
        nc.gpsimd.iota(iota3, pattern=[[-128, 3], [-1, P]], base=128,
                       channel_multiplier=1, allow_small_or_imprecise_dtypes=True)
        ones_t = consts.tile([P, P], F16)
        nc.vector.memset(ones_t, 1.0)
        ones1 = consts.tile([P, 1], F16)
        nc.vector.memset(ones1, 1.0)

        # ACT func-table preload so the 1.3us LoadActFuncSet doesn't land in
        # the middle of sample 0's chain.
        warm = consts.tile([P, 1], F32)
        nc.scalar.activation(out=warm, in_=ones1, func=AF.Identity,
                             bias=0.0, scale=1.0)

        # y_pad tiles allocated once; zero borders persist across samples
        # (color stage only writes the interior columns). Both borders of a
        # tile are cleared in ONE strided memset: dims [c, kt, border, col]
        # with the border dim stepping 0 -> W+PADL.
        y_tiles = []
        for i in range(NY):
            y_t = ypads.tile([P, C, KT, WPAD], F16, tag=f"ypad{i}")
            left = y_t[:, :, :, 0:PADL]
            borders = bass.AP(
                tensor=left.tensor,
                offset=left.offset,
                ap=list(left.ap)[:-1] + [[W + PADL, 2], [1, PADL]],
            )
            nc.gpsimd.memset(borders, 0.0)
            y_tiles.append(y_t)

        # one PE register per sample for the W-shift window (loaded up
        # front so registers are never reused/clobbered across samples)
        _, tyvals = nc.values_load_multi_w_load_instructions(
            scal_sb[0:1, :, SC_TYOFF:SC_TYOFF + 1].bitcast(I32),
            engines=(ET.PE,),
            min_val=0, max_val=2 * SHIFT,
            skip_runtime_bounds_check=True,
        )

        pending = None  # (z_tile, s) awaiting cutout fixups + store

        def flush_pending():
            nonlocal pending
            if pending is None:
                return
            z_t, s = pending
            pending = None
            # registers for the band offsets (DVE)
            _, vals = nc.values_load_multi_w_load_instructions(
                scal_sb[0:1, s, SC_CY0:SC_CY15 + 1].bitcast(I32),
                engines=(ET.DVE,),
                min_val=0, max_val=W - BW2,
                skip_runtime_bounds_check=True,
            )
            cy0v, cy15v = vals
            for mt in range(KT):
                for cyv, bw in ((cy0v, BW1), (cy15v, BW2)):
                    nc.vector.tensor_scalar(
                        out=z_t[:, :, mt, bass.ds(cyv, bw)],
                        in0=z_t[:, :, mt, bass.ds(cyv, bw)],
                        scalar1=rinv_sb[:, s, mt:mt + 1], scalar2=None,
                        op0=OP.mult,
                    )
            z_dst = out_d[s].rearrange("c (kt p) w -> p c kt w", p=P)
            nc.sync.dma_start(out=z_dst, in_=z_t)

        for s in range(B_LOC):
            xs = x_tiles[s]  # [P, C, KT, W] f16

            # ---- S column sums straight off x (PE; free in the cost model,
            # and it runs concurrently with the mc3 adds) ----
            tpz = ptp.tile([P, 2], F32)
            chunks = [(c, kt, j) for c in range(C) for kt in range(KT)
                      for j in range(2)]
            for i, (c, kt, j) in enumerate(chunks):
                nc.tensor.matmul(
                    out=tpz[:, 0:1],
                    lhsT=xs[:, c, kt, j * P:(j + 1) * P],
                    rhs=ones1,
                    start=(i == 0), stop=(i == len(chunks) - 1),
                )

            # ---- mc3 = x0 + x1 (Pool) + x2 (DVE) ----
            # (first two samples fully on DVE: Pool is busy with startup)
            mc3a = map_.tile([P, KT, W], F16)
            if s < 2:
                nc.vector.tensor_add(mc3a, xs[:, 0], xs[:, 1])
            else:
                nc.gpsimd.tensor_add(mc3a, xs[:, 0], xs[:, 1])
            mc3 = mcp.tile([P, KT, W], F16)
            nc.vector.tensor_add(mc3, mc3a, xs[:, 2])

            sA = sap.tile([P, 1], F16)
            nc.scalar.activation(out=sA, in_=tpz[:, 0:1], func=AF.Copy,
                                 bias=0.0, scale=1.0)
            nc.tensor.matmul(out=tpz[:, 1:2], lhsT=ones_t, rhs=sA,
                             start=True, stop=True)

            # ---- D' = (Cp/As)*S + b/As (ACT), t' = (Bp/As)*mc3 + D' (DVE) ----
            D_t = dp.tile([P, 1], F32)
            nc.scalar.activation(out=D_t, in_=tpz[:, 1:2], func=AF.Identity,
                                 bias=sc(s, SC_BA), scale=sc(s, SC_CPA))
            nc.vector.tensor_scalar(
                out=mc3, in0=mc3, scalar1=sc(s, SC_AP),
                scalar2=D_t[:, 0:1], op0=OP.mult, op1=OP.add,
            )

            # ---- As-scaled shift-matrix tile (one fused DVE op) ----
            cmp_t = cmpp.tile([P, 3, P], F16)
            nc.vector.tensor_scalar(
                out=cmp_t, in0=iota3, scalar1=sc(s, SC_TX),
                scalar2=sc(s, SC_AS), op0=OP.is_equal, op1=OP.mult,
            )

            # ---- color: y_c = x_c + t' ----
            y_t = y_tiles[s % NY]
            for c in range(C):
                nc.vector.tensor_add(y_t[:, c, :, PADL:PADL + W], xs[:, c], mc3)

            # cutout fixups + store for the previous sample (keeps DVE from
            # stalling on this sample's evictions)
            flush_pending()

            # ---- H-shift matmuls (W-shift folded in: the rhs reads the
            # dynamic 256-wide window of the padded y, so psum tiles are
            # half as wide and three of them fit in PSUM) ----
            tyv = tyvals[s]
            pzm = []
            for mt in range(KT):
                pz_t = pzp.tile([P, C, W], F32, tag="pz")
                pzm.append(pz_t)
                # c outer: each psum accumulation chain stays contiguous
                # (interleaved start/stop groups miscompile)
                for c in range(C):
                    for kt in range(KT):
                        nc.tensor.matmul(
                            out=pz_t[:, c, :],
                            lhsT=cmp_t[:, mt - kt + 1, :],
                            rhs=y_t[:, c, kt, bass.ds(tyv, W)],
                            start=(kt == 0), stop=(kt == KT - 1),
                        )

            # ---- eviction: plain copy (ACT) ----
            z_t = zp.tile([P, C, KT, W], F16)
            for mt in range(KT):
                nc.scalar.activation(
                    out=z_t[:, :, mt, :],
                    in_=pzm[mt][:, :, :],
                    func=AF.Copy, bias=0.0, scale=1.0,
                )
            pending = (z_t, s)

        flush_pending()


def host_scalars(r_bright, r_sat, r_con, t_x, t_y, off_x, off_y):
    """Per-sample scalar table [B, NSCAL] float32 (int cols bit-cast)."""
    rb = r_bright.reshape(B).astype(np.float64)
    rs = r_sat.reshape(B).astype(np.float64)
    rc = r_con.reshape(B).astype(np.float64)
    txi = t_x.reshape(B).astype(np.int64) - SHIFT   # in [-32, 32]
    tyi = t_y.reshape(B).astype(np.int64) - SHIFT
    oy = off_y.reshape(B).astype(np.int64)

    k = rc + 0.5
    s2 = 2.0 * rs
    A = k * s2
    As = np.maximum(A, A_EPS)
    Bp = k * (1.0 - s2) / 3.0
    Cp = (1.0 - k) / (3.0 * H * W)
    b = rb - 0.5
    cy0 = np.maximum(0, oy - CUT // 2)
    cy1 = np.minimum(W, oy + CUT // 2 + 1)

    tab = np.zeros((B, NSCAL), np.float32)
    tab[:, SC_TX] = txi.astype(np.float32)
    tab[:, SC_AS] = As.astype(np.float32)
    tab[:, SC_AP] = (Bp / As).astype(np.float32)
    tab[:, SC_CPA] = (Cp / As).astype(np.float32)
    tab[:, SC_BA] = (b / As).astype(np.float32)
    tab[:, SC_TYOFF] = (tyi + SHIFT).astype(np.int32).view(np.float32)
    tab[:, SC_CY0] = cy0.astype(np.int32).view(np.float32)
    tab[:, SC_CY15] = (cy1 - BW2).astype(np.int32).view(np.float32)
    return tab


def host_rinv(off_x):
    """[P, B, KT] row-indicator complement: 0 inside the cut rows, else 1."""
    ox = off_x.reshape(B).astype(np.int64)
    rx0 = np.maximum(0, ox - CUT // 2)
    rx1 = np.minimum(H, ox + CUT // 2 + 1)
    rows = (np.arange(P)[:, None, None]
            + P * np.arange(KT)[None, None, :])          # [P, 1, KT]
    inside = (rows >= rx0[None, :, None]) & (rows < rx1[None, :, None])
    return np.where(inside, 0.0, 1.0).astype(np.float32)  # [P, B, KT]


def make_in_maps(x, r_bright, r_sat, r_con, t_x, t_y, off_x, off_y):
    tab = host_scalars(r_bright, r_sat, r_con, t_x, t_y, off_x, off_y)
    rinv = host_rinv(off_x)
    x = np.ascontiguousarray(x, dtype=np.float32).astype(np.float16)
    in_maps = []
    for cid in range(N_CORES):
        lo, hi = cid * B_LOC, (cid + 1) * B_LOC
        in_maps.append({
            "x": x[lo:hi],
            "scal": tab[lo:hi],
            "rinv": np.ascontiguousarray(rinv[:, lo:hi]),
        })
    return in_maps


def kernel(x, r_bright, r_sat, r_con, t_x, t_y, off_x, off_y):
    x, r_bright, r_sat, r_con, t_x, t_y, off_x, off_y = (
        np.asarray(a) for a in (x, r_bright, r_sat, r_con, t_x, t_y, off_x, off_y)
    )
    nc = build_nc()
    in_maps = make_in_maps(x, r_bright, r_sat, r_con, t_x, t_y, off_x, off_y)
    res = bass_utils.run_bass_kernel_spmd(nc, in_maps, core_ids=list(range(N_CORES)))
    out = np.concatenate([res.results[cid]["out"] for cid in range(N_CORES)], axis=0)
    return out.astype(np.float32)
